# revision 1
# baseline (speedup 1.0000x reference)
"""Multi-head attention (B=2, N=2048, D=768, H=12) on 8 Trainium2 NeuronCores.

v3: sequence-sharded with K/V AllGather (no redundant projections).

Each core c owns batch b=c//4 and query rows q0=(c%4)*512 .. q0+512. It
projects Q/K/V only for its OWN 512 rows, AllGathers the K^T and V shards
within its 4-core batch group, and runs attention for its queries against
all 2048 keys. Key order is permutation-invariant under softmax, so each
core processes its own shard first (from SBUF) and the three remote shards
from the gathered buffer, selected with per-core slot indices passed as
input data and applied via dynamic (register-offset) DMAs -- one program
for all 8 cores.

Schedule highlights:
  - Host packs every input partition-major so each DMA moves >=6KB per
    descriptor (the DMA engines are descriptor-rate-bound at ~3GB/s/engine
    for 1KB descriptors).
  - Scalar engine runs only the softmax exp (~110us, the per-core floor);
    exp starts ~12us in (pair-0 weights stream first).
  - K/Q casts+biases on Vector; reciprocal via reciprocal_approx_fast.
  - AV accumulates 4-chunk windows in PSUM, then f32 SBUF accumulators.
  - Collectives (K-AG then V-AG) overlap the local attention phase.
"""

import sys

sys.path.insert(0, "/opt/trn_rl_repo")

import numpy as np

import concourse.bass as bass
import concourse.mybir as mybir
import concourse.tile as tile
from concourse import bacc
from concourse import bass_utils
from concourse.bass import ts as dslice

B, N, D = 2, 2048, 768
H, DH = 12, 64
NCORES = 8
S = 2048          # keys per batch
SQ = 512          # query rows per core
NPAIR = H // 2    # head pairs
KC = D // 128     # contraction chunks
SCALE = DH ** -0.5
RG = [[0, 1, 2, 3], [4, 5, 6, 7]]

KSLAB = NPAIR * 512          # 3072 cols: K^T own shard, pair-major
VSLAB = 4 * NPAIR * 130      # 3120 cols: V own shard, m-major pair blocks

f32 = mybir.dt.float32
bf16 = mybir.dt.bfloat16
i32 = mybir.dt.int32
ADD = mybir.AluOpType.add
EXP = mybir.ActivationFunctionType.Exp

_CACHE = {}


def _build():
    nc = bacc.Bacc("TRN2", target_bir_lowering=False, debug=False,
                   enable_asserts=False, num_devices=NCORES)
    # host-packed inputs (see make_in_maps)
    xh = nc.dram_tensor("xh", [128, KC * 512], bf16, kind="ExternalInput").ap()
    whq = nc.dram_tensor("whq", [128, NPAIR * KC * 128], bf16,
                         kind="ExternalInput").ap()
    whk = nc.dram_tensor("whk", [128, NPAIR * KC * 128], bf16,
                         kind="ExternalInput").ap()
    whv = nc.dram_tensor("whv", [128, KC * 768], bf16,
                         kind="ExternalInput").ap()
    whp = nc.dram_tensor("whp", [128, KC * 768], bf16,
                         kind="ExternalInput").ap()
    bqkv = nc.dram_tensor("bqkv", [128, 18], f32, kind="ExternalInput").ap()
    bproj = nc.dram_tensor("bproj", [D], f32, kind="ExternalInput").ap()
    # per-partition row indices of the 3 remote slots in the gathered bufs
    slotidx = nc.dram_tensor("slotidx", [128, 3], i32,
                             kind="ExternalInput").ap()
    out = nc.dram_tensor("out", [SQ, D], f32, kind="ExternalOutput").ap()

    with tile.TileContext(nc) as tc:
        from contextlib import ExitStack
        with ExitStack() as stack:
            ep = lambda *a, **k: stack.enter_context(tc.tile_pool(*a, **k))
            consts = ep(name="consts", bufs=1)
            w_pool = ep(name="w_pool", bufs=1)
            k_pool = ep(name="k_pool", bufs=1)
            q_pool = ep(name="q_pool", bufs=1)
            v_pool = ep(name="v_pool", bufs=1)
            pt_pool = ep(name="pt_pool", bufs=16)
            acc_pool = ep(name="acc_pool", bufs=1)
            at_pool = ep(name="at_pool", bufs=1)
            nrm_pool = ep(name="nrm_pool", bufs=2)
            outp = ep(name="outp", bufs=2)
            dram = ep(name="dram", bufs=1, space="DRAM")
            ps_sc = ep(name="ps_sc", bufs=2, space="PSUM")
            ps1 = ep(name="ps1", bufs=2, space="PSUM")
            ps0 = ep(name="ps0", bufs=1, space="PSUM")

            # ---- inputs: pair-0 weights first so scores start early ----
            whk0 = w_pool.tile([128, KC * 128], bf16, name="whk0")
            nc.sync.dma_start(out=whk0, in_=whk[:, 0:KC * 128])
            whq0 = w_pool.tile([128, KC * 128], bf16, name="whq0")
            nc.sync.dma_start(out=whq0, in_=whq[:, 0:KC * 128])
            xh_sb = w_pool.tile([128, KC * 512], bf16, name="xh_sb")
            nc.sync.dma_start(out=xh_sb, in_=xh)
            whkr = w_pool.tile([128, 5 * KC * 128], bf16, name="whkr")
            nc.sync.dma_start(out=whkr, in_=whk[:, KC * 128:])
            whqr = w_pool.tile([128, 5 * KC * 128], bf16, name="whqr")
            nc.sync.dma_start(out=whqr, in_=whq[:, KC * 128:])
            # slot indices for the indirect remote loads
            sidx_sb = consts.tile([128, 3], i32, name="sidx_sb")
            nc.sync.dma_start(out=sidx_sb, in_=slotidx)

            bq_sb = consts.tile([128, 18], f32)
            nc.scalar.dma_start(out=bq_sb, in_=bqkv)
            whv_sb = w_pool.tile([128, KC * 768], bf16, name="whv_sb")
            nc.scalar.dma_start(out=whv_sb, in_=whv)
            # warmup exp to preload the ACT table off the critical path
            warm = consts.tile([1, 8], f32)
            nc.vector.memset(warm, 0.0)
            nc.scalar.activation(warm, warm, EXP)
            whp_sb = w_pool.tile([128, KC * 768], bf16, name="whp_sb")
            nc.scalar.dma_start(out=whp_sb, in_=whp)

            bp_bc = consts.tile([128, D], f32)
            bp_in = bass.AP(tensor=bproj.tensor, offset=bproj.offset,
                            ap=[[0, 128]] + list(bproj.ap))
            nc.gpsimd.dma_start(out=bp_bc, in_=bp_in)

            # weight slice helpers
            def wq_jc(j, c):
                base = (j * KC + c) * 128
                if j == 0:
                    return whq0[:, c * 128:(c + 1) * 128]
                return whqr[:, base - KC * 128: base - KC * 128 + 128]

            def wk_jc(j, c):
                base = (j * KC + c) * 128
                if j == 0:
                    return whk0[:, c * 128:(c + 1) * 128]
                return whkr[:, base - KC * 128: base - KC * 128 + 128]

            xts = [xh_sb[:, c * 512:(c + 1) * 512] for c in range(KC)]

            # ---- persistent tiles ----
            kown = k_pool.tile([128, KSLAB], bf16, name="kown")
            qsl = q_pool.tile([128, KSLAB], bf16, name="qsl")
            vsh = v_pool.tile([128, VSLAB], bf16, name="vsh")
            kvslot = [k_pool.tile([128, KSLAB + VSLAB], bf16,
                                  name=f"kvslot{t}") for t in range(3)]
            kslot = [kv[:, 0:KSLAB] for kv in kvslot]
            vslot = [kv[:, KSLAB:] for kv in kvslot]
            at = [at_pool.tile([128, SQ], bf16, name=f"at{j}", tag=f"at{j}")
                  for j in range(NPAIR)]
            acc = [[acc_pool.tile([65, 512], f32, name=f"acc{j}_{h}",
                                  tag=f"acc{j}_{h}") for h in range(2)]
                   for j in range(NPAIR)]

            # ones columns in the own V slab (remote slabs arrive with ones)
            nc.vector.memset(
                vsh.rearrange("p (m j t h) -> p m j t h",
                              m=4, j=NPAIR, t=2)[:, :, :, :, 64:65], 1.0)

            # DRAM bounce + gathered buffer (K and V slabs side by side)
            kvb = dram.tile([128, KSLAB + VSLAB], bf16, name="kvb")
            kvagg = dram.tile([512, KSLAB + VSLAB], bf16, name="kvagg")

            # ---- emission helpers ----
            def emit_k(j):
                kp = ps1.tile([128, 512], f32, name=f"kp{j}", tag="p1")
                for c in range(KC):
                    nc.tensor.matmul(kp, wk_jc(j, c), xts[c],
                                     start=(c == 0), stop=(c == KC - 1))
                nc.vector.tensor_scalar_add(
                    kown[:, j * 512:(j + 1) * 512], kp, bq_sb[:, 6 + j:7 + j])

            def emit_q(j):
                qp = ps1.tile([128, 512], f32, name=f"qp{j}", tag="p1")
                for c in range(KC):
                    nc.tensor.matmul(qp, wq_jc(j, c), xts[c],
                                     start=(c == 0), stop=(c == KC - 1))
                nc.vector.tensor_scalar_add(
                    qsl[:, j * 512:(j + 1) * 512], qp, bq_sb[:, j:j + 1])

            def emit_v(m):
                vr = vsh.rearrange("p (m j t h) -> p m j t h",
                                   m=4, j=NPAIR, t=2)
                vp1 = ps1.tile([128, 512], f32, name=f"vp1_{m}", tag="p1")
                for c in range(KC):
                    nc.tensor.matmul(vp1,
                                     xts[c][:, m * 128:(m + 1) * 128],
                                     whv_sb[:, c * 768:c * 768 + 512],
                                     start=(c == 0), stop=(c == KC - 1))
                nc.vector.tensor_copy(
                    vr[:, m, 0:4, :, 0:64],
                    vp1.rearrange("p (j t h) -> p j t h", j=4, t=2))
                vp2 = ps1.tile([128, 256], f32, name=f"vp2_{m}", tag="p1")
                for c in range(KC):
                    nc.tensor.matmul(vp2,
                                     xts[c][:, m * 128:(m + 1) * 128],
                                     whv_sb[:, c * 768 + 512:(c + 1) * 768],
                                     start=(c == 0), stop=(c == KC - 1))
                nc.vector.tensor_copy(
                    vr[:, m, 4:6, :, 0:64],
                    vp2.rearrange("p (j t h) -> p j t h", j=2, t=2))

            qt = lambda j: qsl[:, j * 512:(j + 1) * 512]

            def emit_scores(j, ksrc):
                # 4 key chunks of one 512-key shard; strip-concurrent heads
                pts = []
                for m in range(4):
                    kc_ = ksrc[:, j * 512 + m * 128: j * 512 + (m + 1) * 128]
                    sc = ps_sc.tile([128, 1024], f32, name=f"sc{j}_{m}",
                                    tag="sc")
                    nc.tensor.matmul(sc[:, 0:512], kc_[0:64, :],
                                     qt(j)[0:64, :], start=True, stop=True)
                    nc.tensor.matmul(sc[:, 512:1024], kc_[64:128, :],
                                     qt(j)[64:128, :], start=True, stop=True)
                    pt = pt_pool.tile([128, 1024], bf16, name=f"p{j}_{m}",
                                      tag="pt")
                    nc.scalar.activation(pt, sc, EXP, scale=SCALE)
                    pts.append(pt)
                return pts

            def emit_av(j, vsrc, pts, first):
                av_e = ps1.tile([65, 512], f32, name=f"ave{j}", tag="p1")
                av_o = ps1.tile([65, 512], f32, name=f"avo{j}", tag="p1")
                for m in range(4):
                    vb = vsrc[:, m * 780 + j * 130: m * 780 + (j + 1) * 130]
                    nc.tensor.matmul(av_e, vb[:, 0:65], pts[m][:, 0:512],
                                     start=(m == 0), stop=(m == 3))
                    nc.tensor.matmul(av_o, vb[:, 65:130], pts[m][:, 512:1024],
                                     start=(m == 0), stop=(m == 3))
                for h, av in ((0, av_e), (1, av_o)):
                    if first:
                        nc.vector.tensor_copy(acc[j][h], av[0:65, :])
                    else:
                        nc.vector.tensor_tensor(acc[j][h], acc[j][h],
                                                av[0:65, :], ADD)

            pp0 = {}

            def emit_normalize(j):
                sums2 = nrm_pool.tile([1, 1024], f32, name=f"sums{j}",
                                      tag="sums")
                nc.vector.tensor_copy(sums2[0:1, 0:512], acc[j][0][64:65, :])
                nc.vector.tensor_copy(sums2[0:1, 512:1024],
                                      acc[j][1][64:65, :])
                rec = nrm_pool.tile([1, 1024], f32, name=f"rec{j}", tag="rec")
                nc.vector.reciprocal_approx_fast(rec, sums2)
                bc_e = nrm_pool.tile([64, 512], f32, name=f"bce{j}", tag="bce")
                nc.gpsimd.partition_broadcast(bc_e, rec[0:1, 0:512])
                bc_o = nrm_pool.tile([64, 512], f32, name=f"bco{j}", tag="bco")
                nc.gpsimd.partition_broadcast(bc_o, rec[0:1, 512:1024])
                nc.vector.tensor_mul(at[j][0:64, :], acc[j][0][0:64, :], bc_e)
                nc.vector.tensor_mul(at[j][64:128, :], acc[j][1][0:64, :],
                                     bc_o)
                nc.vector.tensor_scalar_add(at[j][0:64, :], at[j][0:64, :],
                                            bq_sb[0:64, 12 + j:13 + j])
                nc.vector.tensor_scalar_add(at[j][64:128, :],
                                            at[j][64:128, :],
                                            bq_sb[64:128, 12 + j:13 + j])
                # m=0 projection chain accumulates c=j as pairs normalize
                if j == 0:
                    pp0["a"] = ps0.tile([128, 512], f32, name="pp0a", tag="pa")
                    pp0["b"] = ps0.tile([128, 256], f32, name="pp0b", tag="pb")
                nc.tensor.matmul(pp0["a"], at[j][:, 0:128],
                                 whp_sb[:, j * 768:j * 768 + 512],
                                 start=(j == 0), stop=(j == NPAIR - 1))
                nc.tensor.matmul(pp0["b"], at[j][:, 0:128],
                                 whp_sb[:, j * 768 + 512:(j + 1) * 768],
                                 start=(j == 0), stop=(j == NPAIR - 1))
                if j == NPAIR - 1:
                    ot0 = outp.tile([128, D], f32, name="ot0", tag="ot")
                    nc.vector.tensor_tensor(ot0[:, 0:512], pp0["a"],
                                            bp_bc[:, 0:512], ADD)
                    nc.vector.tensor_tensor(ot0[:, 512:768], pp0["b"],
                                            bp_bc[:, 512:768], ADD)
                    nc.sync.dma_start(out=out[0:128, :], in_=ot0)

            # ---- phase 1: exp-first, then the combined KV-AG ----
            pts_local = [None] * NPAIR
            for j in (0, 1):
                emit_k(j)
                emit_q(j)
                pts_local[j] = emit_scores(j, kown)
            for j in range(2, NPAIR):
                emit_k(j)
            # own K slab to DRAM as soon as it is complete
            nc.sync.dma_start(out=kvb[:, 0:KSLAB], in_=kown)
            for j in range(2, NPAIR):
                emit_q(j)
                pts_local[j] = emit_scores(j, kown)
            for m in range(4):
                emit_v(m)
            nc.sync.dma_start(out=kvb[:, KSLAB:], in_=vsh)
            nc.gpsimd.collective_compute(
                "AllGather", mybir.AluOpType.bypass, replica_groups=RG,
                ins=[kvb.opt()], outs=[kvagg.opt()])

            # ---- local AV windows (need only own V) ----
            for j in range(NPAIR):
                emit_av(j, vsh, pts_local[j], first=True)

            # remote K/V slabs via per-core indirect row gathers
            for t in range(3):
                nc.gpsimd.indirect_dma_start(
                    out=kvslot[t], out_offset=None, in_=kvagg[:],
                    in_offset=bass.IndirectOffsetOnAxis(
                        ap=sidx_sb[:, t:t + 1], axis=0))

            # ---- remote attention, pair-major ----
            for j in range(NPAIR):
                for t in range(3):
                    pts = emit_scores(j, kslot[t])
                    emit_av(j, vslot[t], pts, first=False)
                emit_normalize(j)

            # ---- remaining output projection (m=1..3) ----
            for m in range(1, 4):
                pp = ps_sc.tile([128, 1024], f32, name=f"pp{m}", tag="sc")
                for c in range(KC):
                    nc.tensor.matmul(pp[:, 0:512],
                                     at[c][:, m * 128:(m + 1) * 128],
                                     whp_sb[:, c * 768:c * 768 + 512],
                                     start=(c == 0), stop=(c == KC - 1))
                for c in range(KC):
                    nc.tensor.matmul(pp[:, 512:768],
                                     at[c][:, m * 128:(m + 1) * 128],
                                     whp_sb[:, c * 768 + 512:(c + 1) * 768],
                                     start=(c == 0), stop=(c == KC - 1))
                ot = outp.tile([128, D], f32, name=f"ot{m}", tag="ot")
                nc.vector.tensor_tensor(ot, pp[:, 0:768], bp_bc[:], ADD)
                nc.sync.dma_start(out=out[m * 128:(m + 1) * 128, :], in_=ot)

    nc.compile()
    return nc


def get_nc():
    if "nc" not in _CACHE:
        _CACHE["nc"] = _build()
    return _CACHE["nc"]


def make_in_maps(x, W_qkv, b_qkv, W_proj, b_proj):
    import ml_dtypes
    bf = ml_dtypes.bfloat16
    x = np.asarray(x, dtype=np.float32).astype(bf)
    W_qkv = np.asarray(W_qkv, dtype=np.float32).astype(bf)
    b_qkv = np.asarray(b_qkv, dtype=np.float32)
    W_proj = np.asarray(W_proj, dtype=np.float32)

    cp = np.ascontiguousarray
    # weights packed partition-major (see _build docstring)
    whq = cp(W_qkv[:, 0:768].reshape(6, 128, 6, 128)
             .transpose(1, 2, 0, 3).reshape(128, 4608))
    whk = cp(W_qkv[:, 768:1536].reshape(6, 128, 6, 128)
             .transpose(1, 2, 0, 3).reshape(128, 4608))
    whv = cp(W_qkv[:, 1536:2304].reshape(6, 128, 768)
             .transpose(1, 0, 2).reshape(128, 4608))
    whp = cp(W_proj.astype(bf).reshape(6, 128, 768)
             .transpose(1, 0, 2).reshape(128, 4608))
    bq2d = cp(b_qkv.reshape(18, 128).T)
    bp = cp(np.asarray(b_proj, dtype=np.float32))

    in_maps = []
    for c in range(NCORES):
        b, qi = c // 4, c % 4
        q0 = qi * SQ
        xh = cp(x[b, q0:q0 + SQ].T.reshape(6, 128, 512)
                .transpose(1, 0, 2).reshape(128, 3072))
        sidx = np.stack([((qi + 1 + t) % 4) * 128 + np.arange(128)
                         for t in range(3)], axis=1).astype(np.int32)
        in_maps.append({"xh": xh, "whq": whq, "whk": whk, "whv": whv,
                        "whp": whp, "bqkv": bq2d, "bproj": bp,
                        "slotidx": sidx})
    return in_maps


def run(in_maps, **kw):
    return bass_utils.run_bass_kernel_spmd(get_nc(), in_maps,
                                           core_ids=list(range(NCORES)), **kw)


def kernel(x, W_qkv, b_qkv, W_proj, b_proj):
    in_maps = make_in_maps(x, W_qkv, b_qkv, W_proj, b_proj)
    res = run(in_maps)
    out = np.empty((B, N, D), dtype=np.float32)
    for c in range(NCORES):
        b, q0 = c // 4, (c % 4) * SQ
        out[b, q0:q0 + SQ] = res.results[c]["out"]
    return out



# revision 4
# speedup vs baseline: 1.3686x; 1.3686x over previous
"""Multi-head attention (B=2, N=2048, D=768, H=12) on 8 Trainium2 NeuronCores.

v4: collective-free (batch x head-group x query-half) sharding.

Core c = (b, hg, qh): batch b = c>>2, head-group hg = (c>>1)&1 (heads
hg*6..hg*6+5), query-half qh = c&1 (rows qh*1024..+1024). Each core
projects Q for its 1024 queries and K/V for its 6 heads over ALL 2048
keys locally -- the small redundant K/V projection replaces v3's 94us
AllGather -- then runs attention for 3 head-pairs x 2 query-tiles with
16-deep PSUM accumulation of AV (ones column gives softmax
denominators), and emits a PARTIAL output projection over its 384
head-dims. The host sums the two head-group partials per (b, qh) block
and adds b_proj (that is the unsharding step): no device collective.

The host packs x^T token-block-major with the core's OWN query blocks
at kb slots 0-1 and the remaining half at slots 2-3; key order is
permutation-invariant under softmax so K/V just use that order, and the
single SPMD program never needs to branch on the core id.

Schedule: pair j+1 K/Q projections and the V blocks are interleaved
into the running pair's chunk loop so ACT never starves; scores use
row-tiled concurrent 64-partition matmul pairs; exp reads 1024-wide f32
PSUM and writes bf16 SBUF.
"""

import sys

sys.path.insert(0, "/opt/trn_rl_repo")

import numpy as np

import concourse.bass as bass
import concourse.mybir as mybir
import concourse.tile as tile
from concourse import bacc
from concourse import bass_utils

B, N, D = 2, 2048, 768
H, DH = 12, 64
NCORES = 8
NPAIR = 3          # head pairs per core
KC = D // 128      # contraction chunks (6)
NKB = 4            # 512-token key blocks
NM = 16            # 128-token key chunks
NQT = 2            # 512-query tiles per core
SCALE = DH ** -0.5

f32 = mybir.dt.float32
bf16 = mybir.dt.bfloat16
EXP = mybir.ActivationFunctionType.Exp

_CACHE = {}


def _build():
    nc = bacc.Bacc("TRN2", target_bir_lowering=False, debug=False,
                   enable_asserts=False, num_devices=NCORES)
    # x^T token-block-major: [128, kb(4) * c(6) * 512]
    xh = nc.dram_tensor("xh", [128, NKB * KC * 512], bf16,
                        kind="ExternalInput").ap()
    # Wq/Wk pair-major: block (j, c) at cols (j*KC + c)*128
    whq = nc.dram_tensor("whq", [128, NPAIR * KC * 128], bf16,
                         kind="ExternalInput").ap()
    whk = nc.dram_tensor("whk", [128, NPAIR * KC * 128], bf16,
                         kind="ExternalInput").ap()
    # Wv chunk-major: [128, c(6) * 384]
    whv = nc.dram_tensor("whv", [128, KC * 384], bf16,
                         kind="ExternalInput").ap()
    # W_proj rows for this head-group, pair-major: [128, j(3) * 768]
    whp = nc.dram_tensor("whp", [128, NPAIR * 768], bf16,
                         kind="ExternalInput").ap()
    # col j: Q bias pair j; 3+j: K bias; 6+j: V bias
    bqkv = nc.dram_tensor("bqkv", [128, 9], f32, kind="ExternalInput").ap()
    out = nc.dram_tensor("out", [1024, D], f32, kind="ExternalOutput").ap()

    with tile.TileContext(nc) as tc:
        from contextlib import ExitStack
        with ExitStack() as stack:
            ep = lambda *a, **k: stack.enter_context(tc.tile_pool(*a, **k))
            consts = ep(name="consts", bufs=1)
            w_pool = ep(name="w_pool", bufs=1)
            k_pool = ep(name="k_pool", bufs=1)
            q_pool = ep(name="q_pool", bufs=1)
            v_pool = ep(name="v_pool", bufs=1)
            pt_pool = ep(name="pt_pool", bufs=8)
            at_pool = ep(name="at_pool", bufs=1)
            nrm_pool = ep(name="nrm_pool", bufs=2)
            outp = ep(name="outp", bufs=2)
            ps_sc = ep(name="ps_sc", bufs=2, space="PSUM")
            ps_av = ep(name="ps_av", bufs=4, space="PSUM")

            # ---- inputs: pair-0 weights + x first ----
            whk0 = w_pool.tile([128, KC * 128], bf16, name="whk0")
            nc.sync.dma_start(out=whk0, in_=whk[:, 0:KC * 128])
            whq0 = w_pool.tile([128, KC * 128], bf16, name="whq0")
            nc.sync.dma_start(out=whq0, in_=whq[:, 0:KC * 128])
            xh_sb = w_pool.tile([128, NKB * KC * 512], bf16, name="xh_sb")
            for kb in range(NKB):
                eng = (nc.sync, nc.scalar, nc.gpsimd, nc.sync)[kb]
                eng.dma_start(out=xh_sb[:, kb * 3072:(kb + 1) * 3072],
                              in_=xh[:, kb * 3072:(kb + 1) * 3072])
            bq_sb = consts.tile([128, 9], f32)
            nc.scalar.dma_start(out=bq_sb, in_=bqkv)
            whv_sb = w_pool.tile([128, KC * 384], bf16, name="whv_sb")
            nc.gpsimd.dma_start(out=whv_sb, in_=whv)
            whkr = w_pool.tile([128, 2 * KC * 128], bf16, name="whkr")
            nc.sync.dma_start(out=whkr, in_=whk[:, KC * 128:])
            whqr = w_pool.tile([128, 2 * KC * 128], bf16, name="whqr")
            nc.sync.dma_start(out=whqr, in_=whq[:, KC * 128:])
            # warmup exp to preload the ACT table off the critical path
            warm = consts.tile([1, 8], f32)
            nc.vector.memset(warm, 0.0)
            nc.scalar.activation(warm, warm, EXP)
            whp_sb = w_pool.tile([128, NPAIR * 768], bf16, name="whp_sb")
            nc.gpsimd.dma_start(out=whp_sb, in_=whp)

            def wq_jc(j, c):
                if j == 0:
                    return whq0[:, c * 128:(c + 1) * 128]
                base = ((j - 1) * KC + c) * 128
                return whqr[:, base:base + 128]

            def wk_jc(j, c):
                if j == 0:
                    return whk0[:, c * 128:(c + 1) * 128]
                base = ((j - 1) * KC + c) * 128
                return whkr[:, base:base + 128]

            def xt(c, kb):
                return xh_sb[:, kb * 3072 + c * 512: kb * 3072 + (c + 1) * 512]

            # ---- persistent tiles ----
            kown = k_pool.tile([128, NPAIR * 2048], bf16, name="kown")
            qsl = q_pool.tile([128, NPAIR * 1024], bf16, name="qsl")
            vsh = v_pool.tile([128, NM * NPAIR * 130], bf16, name="vsh")
            at = [at_pool.tile([128, 1024], bf16, name=f"at{j}", tag=f"at{j}")
                  for j in range(NPAIR)]
            vr = vsh.rearrange("p (m j t h) -> p m j t h", m=NM, j=NPAIR, t=2)
            nc.vector.memset(vr[:, :, :, :, 64:65], 1.0)

            # ---- emission helpers ----
            def emit_k(j, kb):
                kp = ps_sc.tile([128, 512], f32, name=f"kp{j}_{kb}", tag="sc")
                for c in range(KC):
                    nc.tensor.matmul(kp, wk_jc(j, c), xt(c, kb),
                                     start=(c == 0), stop=(c == KC - 1))
                nc.vector.tensor_scalar_add(
                    kown[:, j * 2048 + kb * 512: j * 2048 + (kb + 1) * 512],
                    kp, bq_sb[:, 3 + j:4 + j])

            def emit_q(j, qt):
                # query half's token blocks are packed at kb slots 0 and 1
                qp = ps_sc.tile([128, 512], f32, name=f"qp{j}_{qt}", tag="sc")
                for c in range(KC):
                    nc.tensor.matmul(qp, wq_jc(j, c), xt(c, qt),
                                     start=(c == 0), stop=(c == KC - 1))
                nc.vector.tensor_scalar_add(
                    qsl[:, j * 1024 + qt * 512: j * 1024 + (qt + 1) * 512],
                    qp, bq_sb[:, j:j + 1])

            def emit_v(m):
                vp = ps_sc.tile([128, 384], f32, name=f"vp{m}", tag="sc")
                base = (m // 4) * 3072 + (m % 4) * 128
                for c in range(KC):
                    nc.tensor.matmul(
                        vp, xh_sb[:, base + c * 512: base + c * 512 + 128],
                        whv_sb[:, c * 384:(c + 1) * 384],
                        start=(c == 0), stop=(c == KC - 1))
                nc.vector.tensor_copy(
                    vr[:, m, :, :, 0:64],
                    vp.rearrange("p (j t h) -> p j t h", j=NPAIR, t=2))

            def vslab(m, j, t):
                o = (m * NPAIR + j) * 130 + t * 65
                return vsh[:, o:o + 65]

            def emit_chunk(j, qt, m, av_e, av_o):
                # scores for key chunk m (row-tiled concurrent pair) + exp
                kc_ = kown[:, j * 2048 + m * 128: j * 2048 + (m + 1) * 128]
                q2 = qsl[:, j * 1024 + qt * 512: j * 1024 + (qt + 1) * 512]
                sc = ps_sc.tile([128, 1024], f32, name=f"sc{j}_{qt}_{m}",
                                tag="sc")
                nc.tensor.matmul(sc[:, 0:512], kc_[0:64, :], q2[0:64, :],
                                 start=True, stop=True)
                nc.tensor.matmul(sc[:, 512:1024], kc_[64:128, :],
                                 q2[64:128, :], start=True, stop=True)
                pt = pt_pool.tile([128, 1024], bf16, name=f"p{j}_{qt}_{m}",
                                  tag="pt")
                nc.scalar.activation(pt, sc, EXP, scale=SCALE)
                nc.tensor.matmul(av_e, vslab(m, j, 0), pt[:, 0:512],
                                 start=(m == 0), stop=(m == NM - 1))
                nc.tensor.matmul(av_o, vslab(m, j, 1), pt[:, 512:1024],
                                 start=(m == 0), stop=(m == NM - 1))

            def emit_normalize(j, qt, av_e, av_o):
                sums2 = nrm_pool.tile([1, 1024], f32, name=f"sums{j}_{qt}",
                                      tag="sums")
                nc.vector.tensor_copy(sums2[0:1, 0:512], av_e[64:65, :])
                nc.vector.tensor_copy(sums2[0:1, 512:1024], av_o[64:65, :])
                rec = nrm_pool.tile([1, 1024], f32, name=f"rec{j}_{qt}",
                                    tag="rec")
                nc.vector.reciprocal_approx_fast(rec, sums2)
                bc_e = nrm_pool.tile([64, 512], f32, name=f"bce{j}_{qt}",
                                     tag="bce")
                nc.gpsimd.partition_broadcast(bc_e, rec[0:1, 0:512])
                bc_o = nrm_pool.tile([64, 512], f32, name=f"bco{j}_{qt}",
                                     tag="bco")
                nc.gpsimd.partition_broadcast(bc_o, rec[0:1, 512:1024])
                a_e = at[j][0:64, qt * 512:(qt + 1) * 512]
                a_o = at[j][64:128, qt * 512:(qt + 1) * 512]
                nc.vector.tensor_mul(a_e, av_e[0:64, :], bc_e)
                nc.vector.tensor_mul(a_o, av_o[0:64, :], bc_o)
                nc.vector.tensor_scalar_add(a_e, a_e, bq_sb[0:64, 6 + j:7 + j])
                nc.vector.tensor_scalar_add(a_o, a_o,
                                            bq_sb[64:128, 6 + j:7 + j])

            def emit_proj(qt):
                # partial out projection for this 512-query tile (no bias:
                # host adds b_proj after summing head-group partials)
                for mt in range(4):
                    q0 = qt * 512 + mt * 128
                    pp = ps_sc.tile([128, 768], f32, name=f"pp{qt}_{mt}",
                                    tag="sc")
                    for j in range(NPAIR):
                        nc.tensor.matmul(pp[:, 0:512], at[j][:, q0:q0 + 128],
                                         whp_sb[:, j * 768:j * 768 + 512],
                                         start=(j == 0), stop=(j == NPAIR - 1))
                    for j in range(NPAIR):
                        nc.tensor.matmul(pp[:, 512:768], at[j][:, q0:q0 + 128],
                                         whp_sb[:, j * 768 + 512:
                                                (j + 1) * 768],
                                         start=(j == 0), stop=(j == NPAIR - 1))
                    ot = outp.tile([128, D], f32, name=f"ot{qt}_{mt}",
                                   tag="ot")
                    nc.vector.tensor_copy(ot, pp)
                    nc.sync.dma_start(out=out[q0:q0 + 128, :], in_=ot)

            # ---- main pipeline ----
            # pair-0 projections: K over all 4 key blocks, Q over the half
            for kb in range(NKB):
                emit_k(0, kb)
            for qt in range(NQT):
                emit_q(0, qt)

            # filler work interleaved into the chunk loops, one item per
            # chunk: V blocks during (j=0,qt=0); next pair's K/Q during qt=1
            fillers = {
                (0, 0): [lambda m=m: emit_v(m) for m in range(NM)],
                (0, 1): ([lambda kb=kb: emit_k(1, kb) for kb in range(NKB)]
                         + [lambda qt_=qt_: emit_q(1, qt_)
                            for qt_ in range(NQT)]),
                (1, 1): ([lambda kb=kb: emit_k(2, kb) for kb in range(NKB)]
                         + [lambda qt_=qt_: emit_q(2, qt_)
                            for qt_ in range(NQT)]),
            }

            for j in range(NPAIR):
                for qt in range(NQT):
                    fl = fillers.get((j, qt), [])
                    av_e = ps_av.tile([65, 512], f32, name=f"ave{j}_{qt}",
                                      tag="av")
                    av_o = ps_av.tile([65, 512], f32, name=f"avo{j}_{qt}",
                                      tag="av")
                    for m in range(NM):
                        if m < len(fl):
                            fl[m]()
                        emit_chunk(j, qt, m, av_e, av_o)
                    emit_normalize(j, qt, av_e, av_o)
                    if j == NPAIR - 1:
                        emit_proj(qt)

    nc.compile()
    return nc


def get_nc():
    if "nc" not in _CACHE:
        _CACHE["nc"] = _build()
    return _CACHE["nc"]


def make_in_maps(x, W_qkv, b_qkv, W_proj, b_proj):
    import ml_dtypes
    bf = ml_dtypes.bfloat16
    x = np.asarray(x, dtype=np.float32)
    W_qkv = np.asarray(W_qkv, dtype=np.float32)
    b_qkv = np.asarray(b_qkv, dtype=np.float32)
    W_proj = np.asarray(W_proj, dtype=np.float32)

    cp = np.ascontiguousarray
    per_hg = {}
    for hg in range(2):
        s = hg * 384
        whq = cp(W_qkv[:, s:s + 384].astype(bf).reshape(KC, 128, NPAIR, 128)
                 .transpose(1, 2, 0, 3).reshape(128, NPAIR * KC * 128))
        whk = cp(W_qkv[:, 768 + s:768 + s + 384].astype(bf)
                 .reshape(KC, 128, NPAIR, 128)
                 .transpose(1, 2, 0, 3).reshape(128, NPAIR * KC * 128))
        whv = cp(W_qkv[:, 1536 + s:1536 + s + 384].astype(bf)
                 .reshape(KC, 128, 384).transpose(1, 0, 2)
                 .reshape(128, KC * 384))
        whp = cp(W_proj[s:s + 384, :].astype(bf).reshape(NPAIR, 128, 768)
                 .transpose(1, 0, 2).reshape(128, NPAIR * 768))
        cols = []
        for j in range(NPAIR):
            cols.append(b_qkv[s + j * 128: s + (j + 1) * 128])
        for j in range(NPAIR):
            cols.append(b_qkv[768 + s + j * 128: 768 + s + (j + 1) * 128])
        for j in range(NPAIR):
            cols.append(b_qkv[1536 + s + j * 128: 1536 + s + (j + 1) * 128])
        bq2d = cp(np.stack(cols, axis=1))
        per_hg[hg] = (whq, whk, whv, whp, bq2d)

    # x^T token-block-major, per (b, qh): own query blocks at kb 0-1
    xt_b = {}
    for b in range(B):
        xb = x[b].T.astype(bf).reshape(KC, 128, NKB, 512)  # [c,p,kb,t]
        for qh in range(2):
            order = [2 * qh, 2 * qh + 1, 2 * (1 - qh), 2 * (1 - qh) + 1]
            xt_b[(b, qh)] = cp(xb[:, :, order, :].transpose(1, 2, 0, 3)
                               .reshape(128, NKB * KC * 512))

    in_maps = []
    for c in range(NCORES):
        b, hg, qh = c >> 2, (c >> 1) & 1, c & 1
        whq, whk, whv, whp, bq2d = per_hg[hg]
        in_maps.append({"xh": xt_b[(b, qh)], "whq": whq, "whk": whk,
                        "whv": whv, "whp": whp, "bqkv": bq2d})
    return in_maps


def run(in_maps, **kw):
    return bass_utils.run_bass_kernel_spmd(get_nc(), in_maps,
                                           core_ids=list(range(NCORES)), **kw)


def assemble(results, b_proj):
    out = np.empty((B, N, D), dtype=np.float32)
    bp = np.asarray(b_proj, dtype=np.float32)
    for b in range(B):
        for qh in range(2):
            p0 = results[(b << 2) | (0 << 1) | qh]["out"]
            p1 = results[(b << 2) | (1 << 1) | qh]["out"]
            out[b, qh * 1024:(qh + 1) * 1024] = p0 + p1 + bp
    return out


def kernel(x, W_qkv, b_qkv, W_proj, b_proj):
    in_maps = make_in_maps(x, W_qkv, b_qkv, W_proj, b_proj)
    res = run(in_maps)
    return assemble(res.results, b_proj)


# revision 6
# speedup vs baseline: 1.3880x; 1.0141x over previous
"""Multi-head attention (B=2, N=2048, D=768, H=12) on 8 Trainium2 NeuronCores.

v4: collective-free (batch x head-group x query-half) sharding.

Core c = (b, hg, qh): batch b = c>>2, head-group hg = (c>>1)&1 (heads
hg*6..hg*6+5), query-half qh = c&1 (rows qh*1024..+1024). Each core
projects Q for its 1024 queries and K/V for its 6 heads over ALL 2048
keys locally -- the small redundant K/V projection replaces v3's 94us
AllGather -- then runs attention for 3 head-pairs x 2 query-tiles with
16-deep PSUM accumulation of AV (ones column gives softmax
denominators), and emits a PARTIAL output projection over its 384
head-dims. The host sums the two head-group partials per (b, qh) block
and adds b_proj (that is the unsharding step): no device collective.

The host packs x^T token-block-major with the core's OWN query blocks
at kb slots 0-1 and the remaining half at slots 2-3; key order is
permutation-invariant under softmax so K/V just use that order, and the
single SPMD program never needs to branch on the core id.

Schedule: pair j+1 K/Q projections and the V blocks are interleaved
into the running pair's chunk loop so ACT never starves; scores use
row-tiled concurrent 64-partition matmul pairs; exp reads 1024-wide f32
PSUM and writes bf16 SBUF.
"""

import sys

sys.path.insert(0, "/opt/trn_rl_repo")

import numpy as np

import concourse.bass as bass
import concourse.mybir as mybir
import concourse.tile as tile
from concourse import bacc
from concourse import bass_utils

B, N, D = 2, 2048, 768
H, DH = 12, 64
NCORES = 8
NPAIR = 3          # head pairs per core
KC = D // 128      # contraction chunks (6)
NKB = 4            # 512-token key blocks
NM = 16            # 128-token key chunks
NQT = 2            # 512-query tiles per core
SCALE = DH ** -0.5

f32 = mybir.dt.float32
bf16 = mybir.dt.bfloat16
EXP = mybir.ActivationFunctionType.Exp

_CACHE = {}


def _build():
    nc = bacc.Bacc("TRN2", target_bir_lowering=False, debug=False,
                   enable_asserts=False, num_devices=NCORES)
    # x^T token-block-major: [128, kb(4) * c(6) * 512]
    xh = nc.dram_tensor("xh", [128, NKB * KC * 512], bf16,
                        kind="ExternalInput").ap()
    # Wq/Wk pair-major: block (j, c) at cols (j*KC + c)*128
    whq = nc.dram_tensor("whq", [128, NPAIR * KC * 128], bf16,
                         kind="ExternalInput").ap()
    whk = nc.dram_tensor("whk", [128, NPAIR * KC * 128], bf16,
                         kind="ExternalInput").ap()
    # Wv chunk-major: [128, c(6) * 384]
    whv = nc.dram_tensor("whv", [128, KC * 384], bf16,
                         kind="ExternalInput").ap()
    # W_proj rows for this head-group, pair-major: [128, j(3) * 768]
    whp = nc.dram_tensor("whp", [128, NPAIR * 768], bf16,
                         kind="ExternalInput").ap()
    # col j: Q bias pair j; 3+j: K bias; 6+j: V bias
    bqkv = nc.dram_tensor("bqkv", [128, 9], f32, kind="ExternalInput").ap()
    out = nc.dram_tensor("out", [1024, D], f32, kind="ExternalOutput").ap()

    with tile.TileContext(nc) as tc:
        from contextlib import ExitStack
        with ExitStack() as stack:
            ep = lambda *a, **k: stack.enter_context(tc.tile_pool(*a, **k))
            consts = ep(name="consts", bufs=1)
            w_pool = ep(name="w_pool", bufs=1)
            k_pool = ep(name="k_pool", bufs=1)
            q_pool = ep(name="q_pool", bufs=1)
            v_pool = ep(name="v_pool", bufs=1)
            pt_pool = ep(name="pt_pool", bufs=8)
            at_pool = ep(name="at_pool", bufs=1)
            nrm_pool = ep(name="nrm_pool", bufs=2)
            outp = ep(name="outp", bufs=2)
            ps_sc = ep(name="ps_sc", bufs=2, space="PSUM")
            ps_av = ep(name="ps_av", bufs=2, space="PSUM")

            # ---- inputs: pair-0 weights + x first ----
            whk0 = w_pool.tile([128, KC * 128], bf16, name="whk0")
            nc.sync.dma_start(out=whk0, in_=whk[:, 0:KC * 128])
            whq0 = w_pool.tile([128, KC * 128], bf16, name="whq0")
            nc.sync.dma_start(out=whq0, in_=whq[:, 0:KC * 128])
            xh_sb = w_pool.tile([128, NKB * KC * 512], bf16, name="xh_sb")
            for kb in range(NKB):
                eng = (nc.sync, nc.scalar, nc.gpsimd, nc.sync)[kb]
                eng.dma_start(out=xh_sb[:, kb * 3072:(kb + 1) * 3072],
                              in_=xh[:, kb * 3072:(kb + 1) * 3072])
            bq_sb = consts.tile([128, 9], f32)
            nc.scalar.dma_start(out=bq_sb, in_=bqkv)
            whv_sb = w_pool.tile([128, KC * 384], bf16, name="whv_sb")
            nc.gpsimd.dma_start(out=whv_sb, in_=whv)
            whkr = w_pool.tile([128, 2 * KC * 128], bf16, name="whkr")
            nc.sync.dma_start(out=whkr, in_=whk[:, KC * 128:])
            whqr = w_pool.tile([128, 2 * KC * 128], bf16, name="whqr")
            nc.sync.dma_start(out=whqr, in_=whq[:, KC * 128:])
            # warmup exp to preload the ACT table off the critical path
            warm = consts.tile([1, 8], f32)
            nc.vector.memset(warm, 0.0)
            nc.scalar.activation(warm, warm, EXP)
            whp_sb = w_pool.tile([128, NPAIR * 768], bf16, name="whp_sb")
            nc.gpsimd.dma_start(out=whp_sb, in_=whp)

            def wq_jc(j, c):
                if j == 0:
                    return whq0[:, c * 128:(c + 1) * 128]
                base = ((j - 1) * KC + c) * 128
                return whqr[:, base:base + 128]

            def wk_jc(j, c):
                if j == 0:
                    return whk0[:, c * 128:(c + 1) * 128]
                base = ((j - 1) * KC + c) * 128
                return whkr[:, base:base + 128]

            def xt(c, kb):
                return xh_sb[:, kb * 3072 + c * 512: kb * 3072 + (c + 1) * 512]

            # ---- persistent tiles ----
            kown = k_pool.tile([128, NPAIR * 2048], bf16, name="kown")
            qsl = q_pool.tile([128, NPAIR * 1024], bf16, name="qsl")
            vsh = v_pool.tile([128, NM * NPAIR * 130], bf16, name="vsh")
            at = [at_pool.tile([128, 1024], bf16, name=f"at{j}", tag=f"at{j}")
                  for j in range(NPAIR)]
            vr = vsh.rearrange("p (m j t h) -> p m j t h", m=NM, j=NPAIR, t=2)
            nc.vector.memset(vr[:, :, :, :, 64:65], 1.0)

            # ---- emission helpers ----
            def emit_k(j, kb):
                kp = ps_sc.tile([128, 512], f32, name=f"kp{j}_{kb}", tag="fill", bufs=1)
                for c in range(KC):
                    nc.tensor.matmul(kp, wk_jc(j, c), xt(c, kb),
                                     start=(c == 0), stop=(c == KC - 1))
                nc.vector.tensor_scalar_add(
                    kown[:, j * 2048 + kb * 512: j * 2048 + (kb + 1) * 512],
                    kp, bq_sb[:, 3 + j:4 + j])

            def emit_q(j, qt):
                # query half's token blocks are packed at kb slots 0 and 1
                qp = ps_sc.tile([128, 512], f32, name=f"qp{j}_{qt}", tag="fill", bufs=1)
                for c in range(KC):
                    nc.tensor.matmul(qp, wq_jc(j, c), xt(c, qt),
                                     start=(c == 0), stop=(c == KC - 1))
                nc.vector.tensor_scalar_add(
                    qsl[:, j * 1024 + qt * 512: j * 1024 + (qt + 1) * 512],
                    qp, bq_sb[:, j:j + 1])

            def emit_v(m):
                vp = ps_sc.tile([128, 384], f32, name=f"vp{m}", tag="fill", bufs=1)
                base = (m // 4) * 3072 + (m % 4) * 128
                for c in range(KC):
                    nc.tensor.matmul(
                        vp, xh_sb[:, base + c * 512: base + c * 512 + 128],
                        whv_sb[:, c * 384:(c + 1) * 384],
                        start=(c == 0), stop=(c == KC - 1))
                nc.vector.tensor_copy(
                    vr[:, m, :, :, 0:64],
                    vp.rearrange("p (j t h) -> p j t h", j=NPAIR, t=2))

            def vslab(m, j, t):
                o = (m * NPAIR + j) * 130 + t * 65
                return vsh[:, o:o + 65]

            def emit_chunk(j, qt, m, av_e, av_o):
                # scores for key chunk m (row-tiled concurrent pair) + exp
                kc_ = kown[:, j * 2048 + m * 128: j * 2048 + (m + 1) * 128]
                q2 = qsl[:, j * 1024 + qt * 512: j * 1024 + (qt + 1) * 512]
                sc = ps_sc.tile([128, 1024], f32, name=f"sc{j}_{qt}_{m}",
                                tag="sc")
                nc.tensor.matmul(sc[:, 0:512], kc_[0:64, :], q2[0:64, :],
                                 start=True, stop=True)
                nc.tensor.matmul(sc[:, 512:1024], kc_[64:128, :],
                                 q2[64:128, :], start=True, stop=True)
                pt = pt_pool.tile([128, 1024], bf16, name=f"p{j}_{qt}_{m}",
                                  tag="pt")
                nc.scalar.activation(pt, sc, EXP, scale=SCALE)
                nc.tensor.matmul(av_e, vslab(m, j, 0), pt[:, 0:512],
                                 start=(m == 0), stop=(m == NM - 1))
                nc.tensor.matmul(av_o, vslab(m, j, 1), pt[:, 512:1024],
                                 start=(m == 0), stop=(m == NM - 1))

            def emit_normalize(j, qt, av_e, av_o):
                sums2 = nrm_pool.tile([1, 1024], f32, name=f"sums{j}_{qt}",
                                      tag="sums")
                nc.vector.tensor_copy(sums2[0:1, 0:512], av_e[64:65, :])
                nc.vector.tensor_copy(sums2[0:1, 512:1024], av_o[64:65, :])
                rec = nrm_pool.tile([1, 1024], f32, name=f"rec{j}_{qt}",
                                    tag="rec")
                nc.vector.reciprocal_approx_fast(rec, sums2)
                bc_e = nrm_pool.tile([64, 512], f32, name=f"bce{j}_{qt}",
                                     tag="bce")
                nc.gpsimd.partition_broadcast(bc_e, rec[0:1, 0:512])
                bc_o = nrm_pool.tile([64, 512], f32, name=f"bco{j}_{qt}",
                                     tag="bco")
                nc.gpsimd.partition_broadcast(bc_o, rec[0:1, 512:1024])
                a_e = at[j][0:64, qt * 512:(qt + 1) * 512]
                a_o = at[j][64:128, qt * 512:(qt + 1) * 512]
                nc.vector.tensor_mul(a_e, av_e[0:64, :], bc_e)
                nc.vector.tensor_mul(a_o, av_o[0:64, :], bc_o)
                nc.vector.tensor_scalar_add(a_e, a_e, bq_sb[0:64, 6 + j:7 + j])
                nc.vector.tensor_scalar_add(a_o, a_o,
                                            bq_sb[64:128, 6 + j:7 + j])

            def emit_proj(qt):
                # partial out projection for this 512-query tile (no bias:
                # host adds b_proj after summing head-group partials)
                for mt in range(4):
                    q0 = qt * 512 + mt * 128
                    pa = ps_sc.tile([128, 512], f32, name=f"pa{qt}_{mt}",
                                    tag="fill", bufs=1)
                    pb = ps_sc.tile([128, 256], f32, name=f"pb{qt}_{mt}",
                                    tag="ppb", bufs=1)
                    for j in range(NPAIR):
                        nc.tensor.matmul(pa, at[j][:, q0:q0 + 128],
                                         whp_sb[:, j * 768:j * 768 + 512],
                                         start=(j == 0), stop=(j == NPAIR - 1))
                    for j in range(NPAIR):
                        nc.tensor.matmul(pb, at[j][:, q0:q0 + 128],
                                         whp_sb[:, j * 768 + 512:
                                                (j + 1) * 768],
                                         start=(j == 0), stop=(j == NPAIR - 1))
                    ot = outp.tile([128, D], f32, name=f"ot{qt}_{mt}",
                                   tag="ot")
                    nc.vector.tensor_copy(ot[:, 0:512], pa)
                    nc.vector.tensor_copy(ot[:, 512:768], pb)
                    nc.sync.dma_start(out=out[q0:q0 + 128, :], in_=ot)

            # ---- main pipeline ----
            # pair-0 projections: K over all 4 key blocks, Q over the half
            for kb in range(NKB):
                emit_k(0, kb)
            for qt in range(NQT):
                emit_q(0, qt)

            # filler work interleaved into the chunk loops, one item per
            # chunk: V blocks during (j=0,qt=0); next pair's K/Q during qt=1
            fillers = {
                (0, 0): [lambda m=m: emit_v(m) for m in range(NM)],
                (0, 1): ([lambda kb=kb: emit_k(1, kb) for kb in range(NKB)]
                         + [lambda qt_=qt_: emit_q(1, qt_)
                            for qt_ in range(NQT)]),
                (1, 1): ([lambda kb=kb: emit_k(2, kb) for kb in range(NKB)]
                         + [lambda qt_=qt_: emit_q(2, qt_)
                            for qt_ in range(NQT)]),
            }

            for j in range(NPAIR):
                for qt in range(NQT):
                    fl = fillers.get((j, qt), [])
                    av_e = ps_av.tile([65, 512], f32, name=f"ave{j}_{qt}",
                                      tag="av")
                    av_o = ps_av.tile([65, 512], f32, name=f"avo{j}_{qt}",
                                      tag="av")
                    for m in range(NM):
                        if m < len(fl):
                            fl[m]()
                        emit_chunk(j, qt, m, av_e, av_o)
                    emit_normalize(j, qt, av_e, av_o)
                    if j == NPAIR - 1:
                        emit_proj(qt)

    nc.compile()
    return nc


def get_nc():
    if "nc" not in _CACHE:
        _CACHE["nc"] = _build()
    return _CACHE["nc"]


def make_in_maps(x, W_qkv, b_qkv, W_proj, b_proj):
    import ml_dtypes
    bf = ml_dtypes.bfloat16
    x = np.asarray(x, dtype=np.float32)
    W_qkv = np.asarray(W_qkv, dtype=np.float32)
    b_qkv = np.asarray(b_qkv, dtype=np.float32)
    W_proj = np.asarray(W_proj, dtype=np.float32)

    cp = np.ascontiguousarray
    per_hg = {}
    for hg in range(2):
        s = hg * 384
        whq = cp(W_qkv[:, s:s + 384].astype(bf).reshape(KC, 128, NPAIR, 128)
                 .transpose(1, 2, 0, 3).reshape(128, NPAIR * KC * 128))
        whk = cp(W_qkv[:, 768 + s:768 + s + 384].astype(bf)
                 .reshape(KC, 128, NPAIR, 128)
                 .transpose(1, 2, 0, 3).reshape(128, NPAIR * KC * 128))
        whv = cp(W_qkv[:, 1536 + s:1536 + s + 384].astype(bf)
                 .reshape(KC, 128, 384).transpose(1, 0, 2)
                 .reshape(128, KC * 384))
        whp = cp(W_proj[s:s + 384, :].astype(bf).reshape(NPAIR, 128, 768)
                 .transpose(1, 0, 2).reshape(128, NPAIR * 768))
        cols = []
        for j in range(NPAIR):
            cols.append(b_qkv[s + j * 128: s + (j + 1) * 128])
        for j in range(NPAIR):
            cols.append(b_qkv[768 + s + j * 128: 768 + s + (j + 1) * 128])
        for j in range(NPAIR):
            cols.append(b_qkv[1536 + s + j * 128: 1536 + s + (j + 1) * 128])
        bq2d = cp(np.stack(cols, axis=1))
        per_hg[hg] = (whq, whk, whv, whp, bq2d)

    # x^T token-block-major, per (b, qh): own query blocks at kb 0-1
    xt_b = {}
    for b in range(B):
        xb = x[b].T.astype(bf).reshape(KC, 128, NKB, 512)  # [c,p,kb,t]
        for qh in range(2):
            order = [2 * qh, 2 * qh + 1, 2 * (1 - qh), 2 * (1 - qh) + 1]
            xt_b[(b, qh)] = cp(xb[:, :, order, :].transpose(1, 2, 0, 3)
                               .reshape(128, NKB * KC * 512))

    in_maps = []
    for c in range(NCORES):
        b, hg, qh = c >> 2, (c >> 1) & 1, c & 1
        whq, whk, whv, whp, bq2d = per_hg[hg]
        in_maps.append({"xh": xt_b[(b, qh)], "whq": whq, "whk": whk,
                        "whv": whv, "whp": whp, "bqkv": bq2d})
    return in_maps


def run(in_maps, **kw):
    return bass_utils.run_bass_kernel_spmd(get_nc(), in_maps,
                                           core_ids=list(range(NCORES)), **kw)


def assemble(results, b_proj):
    out = np.empty((B, N, D), dtype=np.float32)
    bp = np.asarray(b_proj, dtype=np.float32)
    for b in range(B):
        for qh in range(2):
            p0 = results[(b << 2) | (0 << 1) | qh]["out"]
            p1 = results[(b << 2) | (1 << 1) | qh]["out"]
            out[b, qh * 1024:(qh + 1) * 1024] = p0 + p1 + bp
    return out


def kernel(x, W_qkv, b_qkv, W_proj, b_proj):
    in_maps = make_in_maps(x, W_qkv, b_qkv, W_proj, b_proj)
    res = run(in_maps)
    return assemble(res.results, b_proj)
